# revision 2
# baseline (speedup 1.0000x reference)
"""Trainium2 Bass kernel v3 for DNAShapeNet — sub-array packed schedule.

v2 streamed every conv pass as full 128x128 matmuls (block-sparse lhsT,
~25% MAC utilization on L1-L3).  v3 decomposes each folded conv layer into
32/64-aligned rectangles of NONZERO blocks and issues them as concurrent
`tile_position` matmuls (measured on HW: a 4x 32x32 packed slot streams
N=512 bf16 in ~110-165ns vs ~216ns for one full matmul).

Per (sample, tile) the PE work drops from 11.5 slots to ~6.4:
  L0 1/4 (4 col-tiles stacked in partition groups, host layout)
  L1 2   (A: two 64x64 rects; B: four 32x32 "perm" blocks)
  L2 3   (alpha: 2 rects; beta: 4 blocks; gamma: 4 sigma+-1 blocks)
  L3 3   (sigma0 full; rect-+rect+; 4 perms)   F1 1/2 (pair-packed)
  F2 1/8 (pair slot, 4 col groups)
Evacuations are FD=1024 (2-bank) relu instructions balanced across
ScalarE and VectorE; F2 staging is one FD=512 copy per pair.

PSUM: PL1/PL2/PL3/PX four [128,1024] bank-pairs; PX time-multiplexes
L0/F1; F2 borrows PL1's idle half early in the period.  One semaphore
inc per PE slot; waits are derived from tracked bank/data state.
"""

import numpy as np

import concourse.bass as bass
import concourse.mybir as mybir
from concourse.bass_utils import run_bass_kernel_spmd

F32 = mybir.dt.float32
BF16 = mybir.dt.bfloat16
U16 = mybir.dt.uint16
RELU = mybir.ActivationFunctionType.Relu
IDENT = mybir.ActivationFunctionType.Identity

EPS = 1e-5
KERNELS = [3, 3, 5, 7]
PADS = [1, 1, 2, 3]
B_FULL, CIN0, S = 128, 4, 8192
N_CORES = 8
B_LOC = B_FULL // N_CORES          # 16 samples per core
SF = S // 4                        # 2048 folded cols per sample
TW = 512
NTF = 4
C = 32

# ---- const table column layout (bf16) ----
C_L0, C_L1R, C_L1P = 0, 128, 192
C_L2A, C_L2B, C_L2C, C_L2D, C_L2E = 224, 288, 320, 352, 384
C_L3F, C_L3R, C_L3P = 416, 544, 608
C_F1, C_F2 = 640, 704
CW = 768
CWM = 16                            # constm f32: bias cols 0..4, affine 5..13
OFF_BIAS, OFF_AFF = 0, 5

# rect: (rb, rs, cb, cs, sigma, colbase)
RECTS = {
    "L1A": [(0, 64, 0, 64, 0, C_L1R), (64, 64, 64, 64, 0, C_L1R)],
    "L1B": [(32, 32, 64, 32, 0, C_L1P), (64, 32, 32, 32, 0, C_L1P),
            (96, 32, 0, 32, -1, C_L1P), (0, 32, 96, 32, 1, C_L1P)],
    "L2a": [(0, 64, 0, 64, 0, C_L2A), (64, 64, 64, 64, 0, C_L2A)],
    "L2b": [(0, 64, 64, 32, 0, C_L2B), (64, 64, 32, 32, 0, C_L2B),
            (32, 32, 96, 32, 0, C_L2C), (64, 32, 0, 32, 0, C_L2C)],
    "L2g": [(64, 64, 0, 32, -1, C_L2D), (0, 64, 96, 32, 1, C_L2D),
            (96, 32, 32, 32, -1, C_L2E), (0, 32, 64, 32, 1, C_L2E)],
    "L3F": [(0, 128, 0, 128, 0, C_L3F)],
    "L3R": [(64, 64, 0, 64, -1, C_L3R), (0, 64, 64, 64, 1, C_L3R)],
    "L3P": [(32, 32, 0, 32, -1, C_L3P), (0, 32, 32, 32, 1, C_L3P),
            (96, 32, 64, 32, -1, C_L3P), (64, 32, 96, 32, 1, C_L3P)],
}
LAYER_GROUPS = {1: ("L1A", "L1B"), 2: ("L2a", "L2b", "L2g"),
                3: ("L3F", "L3R", "L3P")}


def _l0_table(w_eff):
    """24x128 lhsT for the folded first layer (same as v2)."""
    k = w_eff.shape[-1]
    pad = k // 2
    tab = np.zeros((24, 128), np.float64)
    for v in range(4):
        for vp in range(4):
            kk = v - vp + pad
            if 0 <= kk < k:
                tab[4 * v:4 * v + CIN0, 32 * vp:32 * vp + C] = w_eff[:, :, kk].T
    tab[16:16 + CIN0, 0:C] = w_eff[:, :, 0].T        # x[c,4u-1] aux rows
    tab[20:20 + CIN0, 96:96 + C] = w_eff[:, :, 2].T  # x[c,4u+4] aux rows
    return tab


def _fold_params(inp):
    const = np.zeros((128, CW), np.float64)
    constm = np.zeros((128, CWM), np.float64)
    fast, zerob = [], []
    w_effs = []
    cin = CIN0
    for l, k in enumerate(KERNELS):
        w = np.asarray(inp[f"w{l}"], np.float64)
        b = np.asarray(inp[f"b{l}"], np.float64)
        g = np.asarray(inp[f"g{l}"], np.float64)
        bb = np.asarray(inp[f"bb{l}"], np.float64)
        rm = np.asarray(inp[f"rm{l}"], np.float64)
        rv = np.asarray(inp[f"rv{l}"], np.float64)
        sc = g / np.sqrt(rv + EPS)
        t = bb - rm * sc
        is_fast = bool(np.all(sc > 0) and np.all(t == 0.0))
        fast.append(is_fast)
        if is_fast:
            w_eff, bias = w * sc[:, None, None], sc * b
            aff_s, aff_t = np.ones(C), np.zeros(C)
        else:
            w_eff, bias = w, b
            aff_s, aff_t = sc, t
        zerob.append(bool(np.all(bias == 0.0)))
        w_effs.append(w_eff)
        for v in range(4):
            constm[32 * v:32 * v + C, OFF_BIAS + l] = bias
            constm[32 * v:32 * v + C, OFF_AFF + 2 * l] = aff_s
            constm[32 * v:32 * v + C, OFF_AFF + 2 * l + 1] = aff_t
        cin = C

    # L0: 24x128 table replicated at partition offsets 32t
    tab0 = _l0_table(w_effs[0])
    for t in range(4):
        const[32 * t:32 * t + 24, C_L0:C_L0 + 128] = tab0

    # conv layers 1..3 via rects; assert exact coverage
    for l in (1, 2, 3):
        k, pad = KERNELS[l], PADS[l]
        covered = set()
        for gname in LAYER_GROUPS[l]:
            for (rb, rs, cb, cs, sig, cbase) in RECTS[gname]:
                for v in range(rb // 32, (rb + rs) // 32):
                    for vp in range(cb // 32, (cb + cs) // 32):
                        kk = pad + 4 * sig + v - vp
                        if 0 <= kk < k:
                            key = (v, vp, sig)
                            assert key not in covered, (l, key)
                            covered.add(key)
                            const[32 * v:32 * v + C,
                                  cbase + 32 * (vp - cb // 32):
                                  cbase + 32 * (vp - cb // 32) + C] \
                                = w_effs[l][:, :, kk].T
        expect = {(v, vp, sig) for v in range(4) for vp in range(4)
                  for sig in (-1, 0, 1) if 0 <= pad + 4 * sig + v - vp < k}
        assert covered == expect, (l, expect - covered, covered - expect)

    fw1 = np.asarray(inp["fw1"], np.float64)
    fb1 = np.asarray(inp["fb1"], np.float64)
    fw2 = np.asarray(inp["fw2"], np.float64)
    f1zero = bool(np.all(fb1 == 0.0))
    for v in range(4):
        const[32 * v:32 * v + C, C_F1 + 16 * v:C_F1 + 16 * v + 16] = fw1.T
        constm[16 * v:16 * v + 16, OFF_BIAS + 4] = fb1
    for st in range(2):
        for v in range(4):
            const[64 * st + 16 * v:64 * st + 16 * v + 16, C_F2 + 4 * st + v] = fw2[0]
    import ml_dtypes
    return ({"constw": const.astype(ml_dtypes.bfloat16),
             "constm": constm.astype(np.float32)}, fast, zerob, f1zero)


def _host_fold_x(x):
    """x [B,4,8192] -> xf2 [B,128,512]: folded rows of col-tile t at
    partitions 32t (24 rows each: 16 main + 4 aux0 + 4 aux1)."""
    import ml_dtypes
    B = x.shape[0]
    xf = np.zeros((B, 24, SF), np.float32)
    xr = x.reshape(B, CIN0, SF, 4)
    xf[:, 0:16, :] = xr.transpose(0, 3, 1, 2).reshape(B, 16, SF)
    xf[:, 16:20, 1:] = xr[:, :, 0:SF - 1, 3]
    xf[:, 20:24, 0:SF - 1] = xr[:, :, 1:SF, 0]
    xf2 = np.zeros((B, 128, TW), np.float32)
    for t in range(4):
        xf2[:, 32 * t:32 * t + 24, :] = xf[:, :, TW * t:TW * (t + 1)]
    return xf2.astype(ml_dtypes.bfloat16)


def _build_program(fast, zerob, f1zero, rep=1):
    nc = bass.Bass()
    x_h = nc.declare_dram_parameter("xf", [B_LOC, 128, TW], BF16, isOutput=False)
    const_h = nc.declare_dram_parameter("constw", [128, CW], BF16, isOutput=False)
    constm_h = nc.declare_dram_parameter("constm", [128, CWM], F32, isOutput=False)
    out_h = nc.declare_dram_parameter("out", [B_LOC, 4, SF], F32, isOutput=True)

    NS = B_LOC * rep
    NPAIR = NS // 2
    allfast = all(fast) and all(zerob) and f1zero

    from contextlib import ExitStack
    with ExitStack() as st:
        ec = st.enter_context
        XS = [ec(nc.sbuf_tensor(f"XS{i}", [128, TW], BF16)) for i in range(4)]
        Z1 = [ec(nc.sbuf_tensor(f"Z1{i}", [128, SF + 2], BF16)) for i in range(2)]
        Z2 = [ec(nc.sbuf_tensor(f"Z2{i}", [128, SF + 2], BF16)) for i in range(2)]
        Z3 = [ec(nc.sbuf_tensor(f"Z3{i}", [128, SF + 2], BF16)) for i in range(2)]
        Z4 = [ec(nc.sbuf_tensor(f"Z4{i}", [128, SF], BF16)) for i in range(4)]
        Hb = [ec(nc.sbuf_tensor(f"Hb{i}", [128, SF], BF16)) for i in range(2)]
        stg = [ec(nc.sbuf_tensor(f"stg{i}", [128, TW], F32)) for i in range(2)]
        constb = ec(nc.sbuf_tensor("constsb", [128, CW], BF16))
        constmb = ec(nc.sbuf_tensor("constmb", [128, CWM], F32))
        PL1 = ec(nc.psum_tensor("PL1", [128, 2 * TW], F32))
        PL2 = ec(nc.psum_tensor("PL2", [128, 2 * TW], F32))
        PL3 = ec(nc.psum_tensor("PL3", [128, 2 * TW], F32))
        PX = ec(nc.psum_tensor("PX", [128, 2 * TW], F32))
        s_w = ec(nc.semaphore("s_w"))
        s_xa = ec(nc.semaphore("s_xa"))
        s_o = ec(nc.semaphore("s_o"))
        s_pe = ec(nc.semaphore("s_pe"))
        s_eS = ec(nc.semaphore("s_eS"))
        s_eD = ec(nc.semaphore("s_eD"))
        block = ec(nc.Block())

        ZIN = {1: Z1, 2: Z2, 3: Z3}     # conv layer l reads ZIN[l]
        ZOUT = {0: Z1, 1: Z2, 2: Z3}
        PSUM = {1: PL1, 2: PL2, 3: PL3}

        # ---------------- schedule generation ----------------
        # slot: dict(mms=[...], idx)  mm: dict describing one matmul
        # evac: dict(kind, eng, ...)  emitted into ACT/DVE streams
        slots = []                    # PE slot list, in order
        act_ev, dve_ev = [], []       # evac lists per engine, in fire order
        NMEMSET = 12

        ev_records = {}               # key -> ("S"/"D", count_after_this)
        bank_last_drain = {}          # bank key -> evac key
        slot_of = {}                  # producer key -> slot idx

        def conv_need_halves(t, sig):
            if sig == 0:
                return {t // 2}
            if sig == -1:
                return {0} if t <= 1 else ({0, 1} if t == 2 else {1})
            return {0} if t == 0 else ({0, 1} if t == 1 else {1})

        def add_slot(mms, tag=None):
            slots.append({"mms": mms, "tag": tag})
            i = len(slots) - 1
            if tag is not None:
                slot_of[tag] = i
            return i

        def add_evac(eng, kind, key, **kw):
            ev = {"kind": kind, "eng": eng, "key": key, **kw}
            if eng == "S":
                act_ev.append(ev)
                ev_records[key] = ("S", len(act_ev))
            else:
                dve_ev.append(ev)
                ev_records[key] = ("D", NMEMSET + len(dve_ev))
            for b in kw.get("drains", ()):
                bank_last_drain[b] = key
            return ev

        def mm_conv(l, s, t, gname, first_of_tile, last_of_tile):
            """MM descriptors for rect group gname of (layer l, sample s, tile t)."""
            out = []
            rects = RECTS[gname]
            for i, (rb, rs, cb, cs, sig, cbase) in enumerate(rects):
                out.append({
                    "kind": "conv", "l": l, "s": s, "t": t,
                    "rb": rb, "rs": rs, "cb": cb, "cs": cs, "sig": sig,
                    "cbase": cbase,
                    "start": first_of_tile and i == 0,
                    "stop": last_of_tile and i == len(rects) - 1,
                })
            return out

        # data-dependency helpers -------------------------------------
        def dep_prev_layer(l, s, t, sigs):
            """evac keys of previous layer halves needed."""
            halves = set()
            for sig in sigs:
                halves |= conv_need_halves(t, sig)
            return [("E", l - 1, s, h) for h in halves]

        def sigs_of(gname):
            return {r[4] for r in RECTS[gname]}

        # ---------------- period loop ----------------
        # period p: L3 of a=p-1, L2 of b=p, L1 of c=p+1, L0 of d=p+2,
        # F1 tiles of pair m3=(p-3)//2 (tau by parity), F2+ST of pair
        # m5=(p-5)//2 on odd-ish periods.
        pe_waits = []                 # per slot: list of (sem, thr)

        def valid(s):
            return 0 <= s < NS

        def sched_conv_slot(items, tag=None):
            """items: list of (l, s, t, gname, first, last).  Emits one slot
            with combined MMs + computed waits."""
            mms, waits = [], []
            for (l, s, t, gname, first, last) in items:
                if not valid(s):
                    continue
                mms += mm_conv(l, s, t, gname, first, last)
                for dk in dep_prev_layer(l, s, t, sigs_of(gname)):
                    assert dk in ev_records, (dk, l, s, t, gname)
                    waits.append(ev_records[dk])
                if first:
                    bk = ("P", l, t % 2)
                    if bk in bank_last_drain:
                        waits.append(ev_records[bank_last_drain[bk]])
            if not mms:
                return None
            i = add_slot(mms, tag)
            pe_waits.append(waits)
            assert len(pe_waits) == len(slots)
            return i

        for p in range(-2, NS + 6):
            a, b, c, d = p - 1, p, p + 1, p + 2
            # F1 cadence: pair m gets tiles (0,1) in period 2m+3, (2,3) in 2m+4
            if (p - 3) % 2 == 0 and valid(2 * ((p - 3) // 2) + 1) and (p - 3) // 2 >= 0:
                f1m, f1taus = (p - 3) // 2, (0, 1)
            elif (p - 4) % 2 == 0 and (p - 4) // 2 >= 0 and valid(2 * ((p - 4) // 2) + 1):
                f1m, f1taus = (p - 4) // 2, (2, 3)
            else:
                f1m, f1taus = None, None
            f2m = (p - 5) // 2 if ((p - 5) % 2 == 0 and (p - 5) // 2 >= 0
                                   and valid(2 * ((p - 5) // 2) + 1)) else None

            # --- slot 1: sigma0(a, t0)
            sched_conv_slot([(3, a, 0, "L3F", True, False)])
            # --- slot 2: F2(f2m) on PL1 half1
            if f2m is not None:
                waits = []
                for k in range(4):
                    dk = ("EF", f2m, k)
                    assert dk in ev_records, dk
                    waits.append(ev_records[dk])
                bk = ("P", 1, 1)
                if bk in bank_last_drain:
                    waits.append(ev_records[bank_last_drain[bk]])
                mms = [{"kind": "F2", "m": f2m, "t": t, "start": True,
                        "stop": True} for t in range(4)]
                add_slot(mms, ("S_F2", f2m))
                pe_waits.append(waits)
                # ST on ACT right away (drains PL1h1)
                w_sts = [("pe", slot_of[("S_F2", f2m)] + 1)]
                if f2m >= 2:
                    w_sts.append(("o", 16 * 4 * (f2m - 1)))
                add_evac("S", "ST", ("ST", f2m), m=f2m, waits=w_sts,
                         drains=[("P", 1, 1)])
            # --- slot 3: sigma0(a, t1)
            sched_conv_slot([(3, a, 1, "L3F", True, False)])
            # --- slots 4,5: beta1(a,t) + L1A(c,t), t=0,1
            for t in (0, 1):
                sched_conv_slot([(3, a, t, "L3R", False, False),
                                 (1, c, t, "L1A", True, False)])
            # --- slots 6,7: beta2(a,t) + gamma(b,t), t=0,1
            for t in (0, 1):
                sched_conv_slot([(3, a, t, "L3P", False, True),
                                 (2, b, t, "L2g", True, False)])
            if valid(a):
                add_evac("S", "conv", ("E", 3, a, 0), l=3, s=a, h=0,
                         waits=[("pe", len(slots))], drains=[("P", 3, 0), ("P", 3, 1)])
            # --- slot 8: L0(d, t0, t1)
            if valid(d):
                waits = [("xa", 16 * (d + 1))]
                for bb_ in (("PX", 0), ("PX", 1)):
                    if bb_ in bank_last_drain:
                        waits.append(ev_records[bank_last_drain[bb_]])
                mms = [{"kind": "L0", "s": d, "t": t, "start": True,
                        "stop": True} for t in (0, 1)]
                add_slot(mms, ("S_L0", d, 0))
                pe_waits.append(waits)
                add_evac("S", "L0", ("E", 0, d, 0), l=0, s=d, h=0,
                         waits=[("pe", len(slots))], drains=[("PX", 0), ("PX", 1)])
            # --- slots 9,10: L1B(c, t0/t1)
            for t in (0, 1):
                sched_conv_slot([(1, c, t, "L1B", False, True)])
            if valid(c):
                add_evac("D", "conv", ("E", 1, c, 0), l=1, s=c, h=0,
                         waits=[("pe", len(slots))], drains=[("P", 1, 0), ("P", 1, 1)])
            # --- slots 11,12: alpha(b,t)+beta(b,t^1)
            for t in (0, 1):
                sched_conv_slot([(2, b, t, "L2a", False, False),
                                 (2, b, t ^ 1, "L2b", False, True)])
            if valid(b):
                add_evac("D", "conv", ("E", 2, b, 0), l=2, s=b, h=0,
                         waits=[("pe", len(slots))], drains=[("P", 2, 0), ("P", 2, 1)])
            # --- slots 13,14: sigma0(a, t2, t3)
            sched_conv_slot([(3, a, 2, "L3F", True, False)])
            sched_conv_slot([(3, a, 3, "L3F", True, False)])
            # --- slots 15,16: beta1(a,t)+L1A(c,t), t=2,3
            for t in (2, 3):
                sched_conv_slot([(3, a, t, "L3R", False, False),
                                 (1, c, t, "L1A", True, False)])
            # --- slot 17: F1(f1m, tau_x)
            def f1_slot(tau):
                m = f1m
                waits = []
                for ss in (2 * m, 2 * m + 1):
                    dk = ("E", 3, ss, tau // 2)
                    if dk in ev_records:
                        waits.append(ev_records[dk])
                bk = ("PX", tau % 2)
                if bk in bank_last_drain:
                    waits.append(ev_records[bank_last_drain[bk]])
                mms = [{"kind": "F1", "s": 2 * m + st, "tau": tau, "start": True,
                        "stop": True} for st in range(2) if valid(2 * m + st)]
                add_slot(mms, ("S_F1", m, tau))
                pe_waits.append(waits)
                eng = "S" if tau % 2 == 0 else "D"
                add_evac(eng, "EF", ("EF", m, tau), m=m, tau=tau,
                         waits=[("pe", len(slots))], drains=[("PX", tau % 2)])
            if f1taus is not None:
                f1_slot(f1taus[0])
            # --- slots 18,19: beta2(a,t)+gamma(b,t), t=2,3
            for t in (2, 3):
                sched_conv_slot([(3, a, t, "L3P", False, True),
                                 (2, b, t, "L2g", True, False)])
            if valid(a):
                add_evac("S", "conv", ("E", 3, a, 1), l=3, s=a, h=1,
                         waits=[("pe", len(slots))], drains=[("P", 3, 0), ("P", 3, 1)])
            # --- slots 20,21: L1B(c, t2/t3)
            for t in (2, 3):
                sched_conv_slot([(1, c, t, "L1B", False, True)])
            if valid(c):
                add_evac("D", "conv", ("E", 1, c, 1), l=1, s=c, h=1,
                         waits=[("pe", len(slots))], drains=[("P", 1, 0), ("P", 1, 1)])
            # --- slot 22: F1(f1m, tau_y)
            if f1taus is not None:
                f1_slot(f1taus[1])
            # --- slots 23,24: alpha(b,t)+beta(b,t^1), t=2,3
            for t in (2, 3):
                sched_conv_slot([(2, b, t, "L2a", False, False),
                                 (2, b, t ^ 1, "L2b", False, True)])
            if valid(b):
                add_evac("D", "conv", ("E", 2, b, 1), l=2, s=b, h=1,
                         waits=[("pe", len(slots))], drains=[("P", 2, 0), ("P", 2, 1)])
            # --- slot 25: L0(d, t2, t3)
            if valid(d):
                waits = []
                for bb_ in (("PX", 0), ("PX", 1)):
                    if bb_ in bank_last_drain:
                        waits.append(ev_records[bank_last_drain[bb_]])
                mms = [{"kind": "L0", "s": d, "t": t, "start": True,
                        "stop": True} for t in (2, 3)]
                add_slot(mms, ("S_L0", d, 1))
                pe_waits.append(waits)
                add_evac("S", "L0", ("E", 0, d, 1), l=0, s=d, h=1,
                         waits=[("pe", len(slots))], drains=[("PX", 0), ("PX", 1)])

        # fix conv "stop" flags: stop must be on the chronologically last MM
        # per (l,s,t) bank; the schedule above sets stop on L1B/L3P/L2b last
        # rect and start on the first rect of the first group; L2's first
        # writer is gamma (slots 6,7,18,19) and last is L2b -- already set.
        # L2a slots pass first=False; gamma passes first=True.  Check L2b
        # "last": tile t's last writer is the beta in the *second* alpha/beta
        # slot of its half.  We set stop=True on all L2b groups' last rect;
        # harmless duplicate stops are avoided by sim only caring about the
        # final one -- instead make only the later slot's beta carry stop:
        # handled below by post-pass.
        by_tile = {}
        for i, sl in enumerate(slots):
            for j, mm in enumerate(sl["mms"]):
                if mm["kind"] == "conv":
                    by_tile.setdefault((mm["l"], mm["s"], mm["t"]), []).append((i, j))
        for key, lst in by_tile.items():
            # start=True iff this MM's partition region is untouched so far in
            # this tile's accumulation group (per-element has_written reset);
            # stop=True on the chronologically last MM.
            covg = [False] * 4
            for (i, j) in lst:
                mm = slots[i]["mms"][j]
                gs = range(mm["cb"] // 32, (mm["cb"] + mm["cs"]) // 32)
                fresh = all(not covg[g] for g in gs)
                assert fresh or all(covg[g] for g in gs), (key, mm)
                mm["start"] = fresh
                mm["stop"] = False
                for g in gs:
                    covg[g] = True
            i, j = lst[-1]
            slots[i]["mms"][j]["stop"] = True

        # ---------------- emission ----------------
        def bias_ap(col):
            return constmb[:, OFF_BIAS + col:OFF_BIAS + col + 1]

        def aff_ap(col):
            return constmb[:, OFF_AFF + col:OFF_AFF + col + 1]

        @block.sync
        def _(eng):
            eng.dma_start(out=constb[:, :], in_=const_h[:, :]).then_inc(s_w, 16)
            eng.dma_start(out=constmb[:, :], in_=constm_h[:, :]).then_inc(s_w, 16)
            hi = 0
            for s in range(NS):
                if s >= 4:
                    key = ("S_L0", s - 4, 1)
                    thr = slot_of[key] + 1
                    if thr > hi:
                        eng.wait_ge(s_pe, thr)
                        hi = thr
                eng.dma_start(out=XS[s % 4][:, :],
                              in_=x_h[s % B_LOC, :, :]).then_inc(s_xa, 16)
            eng.wait_ge(s_o, 16 * 4 * NPAIR)

        @block.tensor
        def _(eng):
            eng.wait_ge(s_w, 32)
            eng.wait_ge(s_eD, NMEMSET)
            hiw = {}

            def do_waits(waits):
                best = {}
                for (sem, thr) in waits:
                    best[sem] = max(best.get(sem, 0), thr)
                for sem, thr in best.items():
                    if thr > hiw.get(sem, 0):
                        hiw[sem] = thr
                        eng.wait_ge({"S": s_eS, "D": s_eD, "pe": s_pe,
                                     "xa": s_xa, "o": s_o}[sem], thr)

            for i, sl in enumerate(slots):
                do_waits(pe_waits[i])
                mms = sl["mms"]
                for j, mm in enumerate(mms):
                    last = j == len(mms) - 1
                    if mm["kind"] == "conv":
                        l, s, t = mm["l"], mm["s"], mm["t"]
                        rb, rs, cb, cs = mm["rb"], mm["rs"], mm["cb"], mm["cs"]
                        zsrc = ZIN[l][s % 2]
                        lo = 1 + TW * t + mm["sig"]
                        out_ap = PSUM[l][cb:cb + cs,
                                         TW * (t % 2):TW * (t % 2) + TW]
                        r = nc.tensor.matmul(
                            out_ap,
                            constb[rb:rb + rs, mm["cbase"]:mm["cbase"] + cs],
                            zsrc[rb:rb + rs, lo:lo + TW],
                            start=mm["start"], stop=mm["stop"],
                            tile_position=(rb, cb),
                        )
                    elif mm["kind"] == "L0":
                        s, t = mm["s"], mm["t"]
                        r = nc.tensor.matmul(
                            PX[:, TW * (t % 2):TW * (t % 2) + TW],
                            constb[32 * t:32 * t + 24, C_L0:C_L0 + 128],
                            XS[s % 4][32 * t:32 * t + 24, :],
                            start=True, stop=True,
                            tile_position=(32 * t, 0),
                        )
                    elif mm["kind"] == "F1":
                        s, tau = mm["s"], mm["tau"]
                        st_ = s % 2
                        r = nc.tensor.matmul(
                            PX[64 * st_:64 * st_ + 64,
                               TW * (tau % 2):TW * (tau % 2) + TW],
                            constb[:, C_F1:C_F1 + 64],
                            Z4[s % 4][:, TW * tau:TW * tau + TW],
                            start=mm["start"], stop=mm["stop"],
                            tile_position=(0, 64 * st_),
                        )
                    else:  # F2
                        m, t = mm["m"], mm["t"]
                        r = nc.tensor.matmul(
                            PL1[32 * t:32 * t + 8, TW:2 * TW],
                            constb[:, C_F2:C_F2 + 8],
                            Hb[m % 2][:, TW * t:TW * t + TW],
                            start=mm["start"], stop=mm["stop"],
                            tile_position=(0, 32 * t),
                        )
                    if last:
                        r.then_inc(s_pe, 1)

        def emit_evacs(eng, evs, sem, my_tag):
            hi = {}

            def w(ev):
                for (s_, thr) in ev["waits"]:
                    if thr > hi.get(s_, 0):
                        hi[s_] = thr
                        eng.wait_ge({"pe": s_pe, "o": s_o}[s_], thr)

            for ev in evs:
                w(ev)
                if ev["kind"] in ("conv", "L0"):
                    l, s, h = ev["l"], ev["s"], ev["h"]
                    src = (PX if l == 0 else PSUM[l])[:, :]
                    if l < 3:
                        dst = ZOUT[l][s % 2][:, 1 + 2 * TW * h:1 + 2 * TW * (h + 1)]
                    else:
                        dst = Z4[s % 4][:, 2 * TW * h:2 * TW * (h + 1)]
                    if fast[l] and zerob[l]:
                        if my_tag == "S":
                            nc.scalar.activation(dst, src, RELU, bias=0.0,
                                                 scale=1.0).then_inc(sem, 1)
                        else:
                            nc.vector.tensor_relu(dst, src).then_inc(sem, 1)
                    else:
                        nc.scalar.activation(dst, src, RELU, bias=bias_ap(l),
                                             scale=1.0).then_inc(sem, 0)
                        nc.scalar.activation(dst, dst, IDENT, bias=aff_ap(2 * l + 1),
                                             scale=aff_ap(2 * l)).then_inc(sem, 1)
                elif ev["kind"] == "EF":
                    m, tau = ev["m"], ev["tau"]
                    src = PX[:, TW * (tau % 2):TW * (tau % 2) + TW]
                    dst = Hb[m % 2][:, TW * tau:TW * tau + TW]
                    if my_tag == "S":
                        nc.scalar.activation(dst, src, RELU,
                                             bias=(0.0 if f1zero else bias_ap(4)),
                                             scale=1.0).then_inc(sem, 1)
                    else:
                        nc.vector.tensor_relu(dst, src).then_inc(sem, 1)
                else:  # ST
                    m = ev["m"]
                    nc.scalar.copy(stg[m % 2][:, :], PL1[:, TW:2 * TW]
                                   ).then_inc(sem, 1)

        @block.scalar
        def _(eng):
            if not allfast:
                eng.wait_ge(s_w, 32)
            emit_evacs(eng, act_ev, s_eS, "S")

        @block.vector
        def _(eng):
            for zb in (Z1, Z2, Z3):
                for i in range(2):
                    nc.vector.memset(zb[i][:, 0:1].bitcast(U16), 0).then_inc(s_eD, 1)
                    nc.vector.memset(zb[i][:, SF + 1:SF + 2].bitcast(U16), 0
                                     ).then_inc(s_eD, 1)
            emit_evacs(eng, dve_ev, s_eD, "D")

        @block.gpsimd
        def _(eng):
            hi = 0
            n_o = 0
            for m in range(NPAIR):
                key = ("ST", m)
                if key not in ev_records:
                    continue
                tag, cnt = ev_records[key]
                assert tag == "S"
                if cnt > hi:
                    eng.wait_ge(s_eS, cnt)
                    hi = cnt
                for t in range(4):
                    eng.dma_start(
                        out=out_h[(2 * m) % B_LOC:(2 * m) % B_LOC + 2, :,
                                  TW * t:TW * (t + 1)],
                        in_=stg[m % 2][32 * t:32 * t + 8, :],
                    ).then_inc(s_o, 16)
                    n_o += 1
            nc._n_out = n_o

    return nc


def _run(inputs, rep=1, trace=False):
    params, fast, zerob, f1zero = _fold_params(inputs)
    nc = _build_program(fast, zerob, f1zero, rep=rep)
    x = np.asarray(inputs["x"], np.float32)
    xf2 = _host_fold_x(x)
    in_maps = []
    for cc in range(N_CORES):
        m = dict(params)
        m["xf"] = np.ascontiguousarray(xf2[cc * B_LOC:(cc + 1) * B_LOC])
        in_maps.append(m)
    res = run_bass_kernel_spmd(nc, in_maps, core_ids=list(range(N_CORES)),
                               trace=trace)
    of = np.concatenate([res.results[c]["out"] for c in range(N_CORES)], axis=0)
    out = of.transpose(0, 2, 1).reshape(B_FULL, S)
    fb2 = np.asarray(inputs["fb2"], np.float32)
    if np.any(fb2 != 0):
        out = out + fb2[0]
    return np.ascontiguousarray(out.astype(np.float32)), res


def kernel(**inputs):
    out, _ = _run(inputs, trace=False)
    return out
